# revision 6
# baseline (speedup 1.0000x reference)
"""Distributed multi-head attention for trn2 (8 NeuronCores).

Problem: B=4, S=1024, H=1024, nh=16, hd=64; mask is all-ones, biases are
zero (both fixed by the problem's input spec), so neither reaches the
device.

Sharding: core c = b*2 + g handles batch b = c//2 and head-group
g = c%2 (8 heads = 512 hidden dims).  Each core computes
  qT,kT = (Wq_g @ x_b.T), (Wk_g @ x_b.T)      [512, 1024]
  v     = x_b @ Wv_g.T                        [1024, 512]
  per head: scoresT = kT_h.T-contract-qT_h    [tk, tq] psum (K=64 row-tiled,
            two heads run concurrently in disjoint 64-row PE groups)
            probsT  = exp(scoresT / 8)        (ACT, bf16 out)
            ctxT_aug = [v_h | 1].T @ probsT   rows 0-63 ctx, row 64-127 rowsum
            ctxT = ctxT_aug[0:64] * (1/rowsum)  (DVE)
  partial_out = ctxT.T @ Wo_g_rows.T          [1024, 1024]
Host sums the two partials of each batch (row-parallel Wo unshard) and
stacks the 4 batches.

Scheduling: emission is interleaved so the PE never waits on the
ACT-bound softmax stream -- qk pairs 1-3 and the v projection are pumped
as filler between score/ctx matmuls, and ctx for pair m-1 overlaps the
scores of pair m.
"""

import sys
from collections import deque

import numpy as np

sys.path.insert(0, "/opt/trn_rl_repo")

import ml_dtypes  # noqa: E402

import concourse.bass as bass  # noqa: E402
import concourse.tile as tile  # noqa: E402
from concourse import bacc, mybir  # noqa: E402
from concourse.bass_utils import run_bass_kernel_spmd  # noqa: E402

S = 1024  # sequence length
H = 1024  # hidden
NH_LOC = 8  # heads per core
HD = 64  # head dim
HG = 512  # hidden dims per core's head group
P = 128  # partitions

F32 = mybir.dt.float32
BF16 = mybir.dt.bfloat16
INPUT_DT = BF16

_CACHE: dict = {}

ALL_PHASES = ("qkv", "attn", "exp", "ctx", "out")


def _build_graph(reps: int = 1, timing: bool = False, phases=ALL_PHASES):
    nc = bacc.Bacc(
        "TRN2", target_bir_lowering=False, debug=False, num_devices=8
    )

    kind = "Internal" if timing else "ExternalInput"
    okind = "Internal" if timing else "ExternalOutput"
    xt_d = nc.dram_tensor("xt", [H, S], INPUT_DT, kind=kind).ap()
    wqt_d = nc.dram_tensor("wqt", [H, HG], INPUT_DT, kind=kind).ap()
    wkt_d = nc.dram_tensor("wkt", [H, HG], INPUT_DT, kind=kind).ap()
    wvt_d = nc.dram_tensor("wvt", [H, HG], INPUT_DT, kind=kind).ap()
    wot_d = nc.dram_tensor("wot", [HG, H], BF16, kind=kind).ap()
    out_d = nc.dram_tensor("out_p", [S, H], F32, kind=okind).ap()
    tok_d = (
        nc.dram_tensor("tok", [1, 4], F32, kind="ExternalOutput").ap()
        if timing
        else None
    )

    with tile.TileContext(nc) as tc:
        with tc.tile_pool(name="inp", bufs=1) as inp:
            tiles = _dma_inputs(tc, inp, xt_d, wqt_d, wkt_d, wvt_d, wot_d)
            # v_aug tiles: per head h, cols h*128..+63 = v, cols +64..+127 = 1.0
            # (64 ones-columns put 64 copies of the softmax denominator on
            # psum rows 64-127).  The ones are static -- set once, outside
            # the timing loop.
            v_sb = [
                inp.tile([P, NH_LOC * P], BF16, tag=f"v{i}", name=f"v{i}")
                for i in range(8)
            ]
            for t in v_sb:
                for h in range(NH_LOC):
                    nc.gpsimd.memset(t[:, h * P + HD : (h + 1) * P], 1.0)

            pre = {}
            if reps != 1:
                if "qkv" not in phases:
                    for m in range(4):
                        for tag in ("qT2", "kT2"):
                            t = inp.tile([P, S], BF16, tag=f"{tag}{m}")
                            nc.gpsimd.memset(t[:], 0.125)
                            pre[f"{tag}{m}"] = t
                    for t in v_sb:
                        nc.gpsimd.memset(t[:], 0.125)
                if "exp" not in phases and "ctx" in phases:
                    for tk in range(8):
                        t = inp.tile([P, S], BF16, tag=f"pb{tk}")
                        nc.gpsimd.memset(t[:], 0.001)
                        pre[f"pb{tk}"] = t
                if "ctx" not in phases and "out" in phases:
                    for m in range(4):
                        t = inp.tile([P, S], BF16, tag=f"ctxT{m}")
                        nc.gpsimd.memset(t[:], 0.125)
                        pre[f"ctxT{m}"] = t

            if reps == 1:
                _compute(tc, tiles, v_sb, out_d, tok_d, phases, pre)
            else:
                hints = (
                    mybir.EngineType.PE,
                    mybir.EngineType.DVE,
                    mybir.EngineType.Pool,
                )
                with tc.For_i(0, reps, 1, hint_engines=hints):
                    _compute(tc, tiles, v_sb, out_d, tok_d, phases, pre)

    nc.compile()
    return nc


def _dma_inputs(tc, inp, xt_d, wqt_d, wkt_d, wvt_d, wot_d):
    # interleave per contraction-chunk so the first qk accumulation chains
    # can start as soon as chunk 0 lands instead of waiting for whole
    # tensors
    nc = tc.nc
    xt, wqt, wkt, wvt = [], [], [], []
    for kc in range(8):
        t = inp.tile([P, S], INPUT_DT, tag=f"xt{kc}", name=f"xt{kc}")
        nc.sync.dma_start(t[:], xt_d[kc * P : (kc + 1) * P, :])
        xt.append(t)
        for lst, d, tag in (
            (wqt, wqt_d, "wqt"), (wkt, wkt_d, "wkt"), (wvt, wvt_d, "wvt")
        ):
            t = inp.tile([P, HG], INPUT_DT, tag=f"{tag}{kc}", name=f"{tag}{kc}")
            nc.sync.dma_start(t[:], d[kc * P : (kc + 1) * P, :])
            lst.append(t)
    wot = []  # 4 x [128, 1024] bf16, rows = local c
    for cc in range(4):
        t = inp.tile([P, H], BF16, tag=f"wot{cc}", name=f"wot{cc}")
        nc.sync.dma_start(t[:], wot_d[cc * P : (cc + 1) * P, :])
        wot.append(t)
    return xt, wqt, wkt, wvt, wot


def _compute(tc, tiles, v_sb, out_d, tok_d=None, phases=ALL_PHASES, pre=None):
    pre = pre or {}
    nc = tc.nc
    xt, wqt, wkt, wvt, wot = tiles
    from contextlib import ExitStack

    do_qkv = "qkv" in phases
    do_attn = "attn" in phases
    do_exp = "exp" in phases
    do_ctx = "ctx" in phases and do_attn
    do_out = "out" in phases

    ctx_stk = ExitStack()
    with ctx_stk:
        acts = ctx_stk.enter_context(tc.tile_pool(name="acts", bufs=1))
        probs_pool = ctx_stk.enter_context(tc.tile_pool(name="probs", bufs=32))
        small = ctx_stk.enter_context(tc.tile_pool(name="small", bufs=4))
        outsb = ctx_stk.enter_context(tc.tile_pool(name="outsb", bufs=4))
        # PSUM budget (8 banks):
        #   pp  [128,1024] x1 = 2 banks  (qk chains; v chunks use half)
        #   scA/scB [128,1024] x1 each = 4 banks (pair-concurrent scores)
        #   pc0/pc1 [128,512] x1 each = 2 banks (ctx accum; out chunks)
        ps_pp = ctx_stk.enter_context(
            tc.tile_pool(name="ps_pp", bufs=1, space="PSUM")
        )
        ps_sc = ctx_stk.enter_context(
            tc.tile_pool(name="ps_sc", bufs=1, space="PSUM")
        )
        ps_c = ctx_stk.enter_context(
            tc.tile_pool(name="ps_c", bufs=1, space="PSUM")
        )

        # persistent activations: qT2/kT2 pair tiles -- partitions 0:64 =
        # head 2m, partitions 64:128 = head 2m+1 (natural projection layout,
        # no padding needed: scores run as two K=64 row-tiled matmuls)
        qT2 = [
            pre.get(f"qT2{m}")
            or acts.tile([P, S], BF16, tag=f"qT2{m}", name=f"qT2{m}")
            for m in range(4)
        ]
        kT2 = [
            pre.get(f"kT2{m}")
            or acts.tile([P, S], BF16, tag=f"kT2{m}", name=f"kT2{m}")
            for m in range(4)
        ]
        ctxT = [
            pre.get(f"ctxT{m}")
            or acts.tile([P, S], BF16, tag=f"ctxT{m}", name=f"ctxT{m}")
            for m in range(4)
        ]

        probs = [[None] * 8 for _ in range(NH_LOC)]
        if not do_exp:
            for h in range(NH_LOC):
                for tk in range(8):
                    probs[h][tk] = pre.get(f"pb{tk}")

        # ---- filler generators (pumped between attention steps) ---------
        def gen_qk(m):
            for w, dst in ((wqt, qT2[m]), (wkt, kT2[m])):
                ps = ps_pp.tile([P, S], F32, tag="pp", name="ps_qk")
                for kc in range(8):
                    for th in range(2):
                        nc.tensor.matmul(
                            ps[:, th * 512 : (th + 1) * 512],
                            w[kc][:, m * P : (m + 1) * P],
                            xt[kc][:, th * 512 : (th + 1) * 512],
                            start=(kc == 0),
                            stop=(kc == 7),
                        )
                    if kc % 2 == 1:
                        yield
                nc.vector.tensor_copy(dst[:], ps[:])
                yield

        def gen_v():
            for tci in range(8):
                # time-shares the ctx psum banks: v finishes during pair 0,
                # ctx accumulation only starts at pair-0 end
                ps = ps_c.tile([P, HG], F32, tag=f"pc{tci % 2}", name="ps_v")
                for kc in range(8):
                    nc.tensor.matmul(
                        ps[:],
                        xt[kc][:, tci * P : (tci + 1) * P],
                        wvt[kc][:],
                        start=(kc == 0),
                        stop=(kc == 7),
                    )
                    if kc % 4 == 3:
                        yield
                for h in range(NH_LOC):
                    # alternate copy engine: DVE and ACT both reach PSUM
                    src = ps[:, h * HD : (h + 1) * HD]
                    dst = v_sb[tci][:, h * P : h * P + HD]
                    if h % 2 == 0:
                        nc.vector.tensor_copy(dst, src)
                    else:
                        nc.scalar.activation(
                            dst, src, mybir.ActivationFunctionType.Copy
                        )
                yield

        def gen_ctx(h):
            m, hh = h // 2, h % 2
            pch = [
                ps_c.tile([P, 512], F32, tag=f"pc{th}", name=f"pc{th}")
                for th in range(2)
            ]
            for tk in range(8):
                for th in range(2):
                    nc.tensor.matmul(
                        pch[th][:],
                        v_sb[tk][:, h * P : (h + 1) * P],
                        probs[h][tk][:, th * 512 : (th + 1) * 512],
                        start=(tk == 0),
                        stop=(tk == 7),
                    )
                yield
            for th in range(2):
                rp = small.tile([HD, 512], F32, tag="recip", name="rp")
                nc.vector.reciprocal(rp[:], pch[th][64:128, :])
                nc.vector.tensor_tensor(
                    ctxT[m][hh * HD : (hh + 1) * HD, th * 512 : (th + 1) * 512],
                    pch[th][0:HD, :],
                    rp[:],
                    mybir.AluOpType.mult,
                )
            yield

        fillers = deque()
        if do_qkv:
            fillers.append(("qk1", gen_qk(1)))
            fillers.append(("v", gen_v()))
            fillers.append(("qk2", gen_qk(2)))
            fillers.append(("qk3", gen_qk(3)))
        done_fillers = set()
        ctx_tasks = deque()

        def pump_filler(n):
            for _ in range(n):
                while fillers:
                    name, g = fillers[0]
                    try:
                        next(g)
                        break
                    except StopIteration:
                        done_fillers.add(name)
                        fillers.popleft()
                else:
                    return

        def drain_filler(name):
            while fillers and name not in done_fillers:
                pump_filler(1)

        def pump_ctx(n):
            for _ in range(n):
                while ctx_tasks:
                    try:
                        next(ctx_tasks[0])
                        break
                    except StopIteration:
                        ctx_tasks.popleft()
                else:
                    return

        # ---- lead-in: first qk pair ------------------------------------
        if do_qkv:
            for _ in gen_qk(0):
                pass

        # ---- attention: pair loop with interleaved ctx + filler --------
        if do_attn:
            for m in range(4):
                # scores of pair m read qT2[m]/kT2[m]; ctx of pair m-1
                # reads v_sb -- their producers must be emitted first
                if do_qkv and m >= 1:
                    drain_filler("v")
                    drain_filler(f"qk{m}")
                for tk in range(8):
                    psA = ps_sc.tile([P, S], F32, tag="scA", name="scA")
                    psB = ps_sc.tile([P, S], F32, tag="scB", name="scB")
                    for th in range(2):
                        nc.tensor.matmul(
                            psA[:, th * 512 : (th + 1) * 512],
                            kT2[m][0:HD, tk * P : (tk + 1) * P],
                            qT2[m][0:HD, th * 512 : (th + 1) * 512],
                            start=True,
                            stop=True,
                        )
                        nc.tensor.matmul(
                            psB[:, th * 512 : (th + 1) * 512],
                            kT2[m][HD:P, tk * P : (tk + 1) * P],
                            qT2[m][HD:P, th * 512 : (th + 1) * 512],
                            start=True,
                            stop=True,
                        )
                    if do_exp:
                        for h, ps in ((2 * m, psA), (2 * m + 1, psB)):
                            pb = probs_pool.tile(
                                [P, S], BF16, tag="pb", name="pb"
                            )
                            nc.scalar.activation(
                                pb[:], ps[:],
                                mybir.ActivationFunctionType.Exp,
                                scale=0.125,
                            )
                            probs[h][tk] = pb
                    pump_ctx(2)
                    pump_filler(5 if m == 0 else 2)
                if do_ctx:
                    ctx_tasks.extend([gen_ctx(2 * m), gen_ctx(2 * m + 1)])
                pump_ctx(2)

        # drain leftovers
        pump_filler(1000)
        pump_ctx(1000)

        # ---- output projection -----------------------------------------
        if do_out:
            for tci in range(8):
                oa = outsb.tile([P, H], F32, tag="oa", name="oa")
                for ho in range(2):
                    ps = ps_c.tile(
                        [P, 512], F32, tag=f"pc{ho}", name=f"pc{ho}"
                    )
                    for cc in range(4):
                        nc.tensor.matmul(
                            ps[:],
                            ctxT[cc][:, tci * P : (tci + 1) * P],
                            wot[cc][:, ho * 512 : (ho + 1) * 512],
                            start=(cc == 0),
                            stop=(cc == 3),
                        )
                    dst = oa[:, ho * 512 : (ho + 1) * 512]
                    if ho == 0:
                        nc.vector.tensor_copy(dst, ps[:])
                    else:
                        nc.scalar.activation(
                            dst, ps[:], mybir.ActivationFunctionType.Copy
                        )
                nc.sync.dma_start(
                    out_d[tci * P : (tci + 1) * P, :], oa[:]
                )

        if tok_d is not None:
            tk_t = small.tile([1, 4], F32, tag="tok")
            nc.gpsimd.memset(tk_t[:], 0.0)
            nc.sync.dma_start(tok_d[:], tk_t[:])


def _get_nc():
    if "nc" not in _CACHE:
        _CACHE["nc"] = _build_graph()
    return _CACHE["nc"]


def kernel(x, mask, Wq, bq, Wk, bk, Wv, bv, Wo, bo):
    x = np.asarray(x, dtype=np.float32)
    Wq = np.asarray(Wq, dtype=np.float32)
    Wk = np.asarray(Wk, dtype=np.float32)
    Wv = np.asarray(Wv, dtype=np.float32)
    Wo = np.asarray(Wo, dtype=np.float32)

    nc = _get_nc()
    bf = ml_dtypes.bfloat16
    in_maps = []
    for c in range(8):
        b, g = c // 2, c % 2
        sl = slice(g * HG, (g + 1) * HG)
        in_maps.append(
            {
                "xt": np.ascontiguousarray(x[b].T.astype(bf)),
                "wqt": np.ascontiguousarray(Wq[sl, :].T.astype(bf)),
                "wkt": np.ascontiguousarray(Wk[sl, :].T.astype(bf)),
                "wvt": np.ascontiguousarray(Wv[sl, :].T.astype(bf)),
                "wot": np.ascontiguousarray(Wo[:, sl].T.astype(bf)),
            }
        )
    res = run_bass_kernel_spmd(
        nc, in_maps, core_ids=list(range(8)), **_CACHE.get("run_kwargs", {})
    )
    _CACHE["last_result"] = res
    outs = [res.results[c]["out_p"] for c in range(8)]
    return np.stack(
        [outs[2 * b] + outs[2 * b + 1] for b in range(4)]
    ).astype(np.float32)


# revision 7
# speedup vs baseline: 1.1638x; 1.1638x over previous
"""Distributed multi-head attention for trn2 (8 NeuronCores).

Problem: B=4, S=1024, H=1024, nh=16, hd=64; mask is all-ones, biases are
zero (both fixed by the problem's input spec), so neither reaches the
device.

Sharding: core c = b*2 + g handles batch b = c//2 and head-group
g = c%2 (8 heads = 512 hidden dims).  Each core computes
  qT,kT = (Wq_g @ x_b.T), (Wk_g @ x_b.T)      [512, 1024]
  v     = x_b @ Wv_g.T                        [1024, 512]
  per head: scoresT = kT_h.T-contract-qT_h    [tk, tq] psum
            probsT  = exp(scoresT / 8)        (ACT, bf16 out)
            ctxT_aug = [v_h | 1].T @ probsT   rows 0-63 ctx, rows 64-127 rowsum
            ctxT = ctxT_aug[0:64] * (1/rowsum)  (DVE)
  partial_out = ctxT.T @ Wo_g_rows.T          [1024, 1024]
Host sums the two partials of each batch (row-parallel Wo unshard) and
stacks the 4 batches.

Key tricks:
- kT is stored PAIR-PACKED ([128,1024]: rows 0:64 head 2m, 64:128 head
  2m+1) and used as a shared K=128 stationary for both heads' scores.
  The per-head selection happens on the MOVING side: qE/qO tiles have
  the other head's 64 rows statically zeroed (memset once, outside the
  timing loop), so zero-rows x junk-weights contribute nothing.  Full
  K=128 keeps FWL on and needs no per-rep memsets.
- scheduling: qk pairs 1-3 and the v projection are emitted as filler
  between attention steps so the PE stays busy while ACT drains the
  softmax; ctx for pair m-1 overlaps the scores of pair m; psum->sbuf
  copies alternate between DVE and ACT.
"""

import sys
from collections import deque

import numpy as np

sys.path.insert(0, "/opt/trn_rl_repo")

import ml_dtypes  # noqa: E402

import concourse.bass as bass  # noqa: E402
import concourse.tile as tile  # noqa: E402
from concourse import bacc, mybir  # noqa: E402
from concourse.bass_utils import run_bass_kernel_spmd  # noqa: E402

S = 1024  # sequence length
H = 1024  # hidden
NH_LOC = 8  # heads per core
HD = 64  # head dim
HG = 512  # hidden dims per core's head group
P = 128  # partitions

F32 = mybir.dt.float32
BF16 = mybir.dt.bfloat16
INPUT_DT = BF16

_CACHE: dict = {}

ALL_PHASES = ("qkv", "attn", "exp", "ctx", "out")


def _build_graph(reps: int = 1, timing: bool = False, phases=ALL_PHASES):
    nc = bacc.Bacc(
        "TRN2", target_bir_lowering=False, debug=False, num_devices=8
    )

    kind = "Internal" if timing else "ExternalInput"
    okind = "Internal" if timing else "ExternalOutput"
    xt_d = nc.dram_tensor("xt", [H, S], INPUT_DT, kind=kind).ap()
    wqt_d = nc.dram_tensor("wqt", [H, HG], INPUT_DT, kind=kind).ap()
    wkt_d = nc.dram_tensor("wkt", [H, HG], INPUT_DT, kind=kind).ap()
    wvt_d = nc.dram_tensor("wvt", [H, HG], INPUT_DT, kind=kind).ap()
    wot_d = nc.dram_tensor("wot", [HG, H], BF16, kind=kind).ap()
    out_d = nc.dram_tensor("out_p", [S, H], F32, kind=okind).ap()
    tok_d = (
        nc.dram_tensor("tok", [1, 4], F32, kind="ExternalOutput").ap()
        if timing
        else None
    )

    with tile.TileContext(nc) as tc:
        with tc.tile_pool(name="inp", bufs=1) as inp:
            tiles = _dma_inputs(tc, inp, xt_d, wqt_d, wkt_d, wvt_d, wot_d)

            # ---- persistent tiles with static parts (set once) ----------
            persist = {}
            # v_aug: per head h, cols h*128..+63 = v, cols +64..+127 = 1.0
            persist["v_sb"] = [
                inp.tile([P, NH_LOC * P], BF16, tag=f"v{i}", name=f"v{i}")
                for i in range(8)
            ]
            for t in persist["v_sb"]:
                for h in range(NH_LOC):
                    nc.gpsimd.memset(t[:, h * P + HD : (h + 1) * P], 1.0)
            # padded moving-q tiles: qE rows 64:128 = 0, qO rows 0:64 = 0
            persist["qE"] = [
                inp.tile([P, S], BF16, tag=f"qE{m}", name=f"qE{m}")
                for m in range(4)
            ]
            persist["qO"] = [
                inp.tile([P, S], BF16, tag=f"qO{m}", name=f"qO{m}")
                for m in range(4)
            ]
            for m in range(4):
                nc.gpsimd.memset(persist["qE"][m][HD:P, :], 0.0)
                nc.gpsimd.memset(persist["qO"][m][0:HD, :], 0.0)
            # pair-packed k (shared stationary) and ctx output tiles
            persist["kT2"] = [
                inp.tile([P, S], BF16, tag=f"kT2{m}", name=f"kT2{m}")
                for m in range(4)
            ]
            persist["ctxT"] = [
                inp.tile([P, S], BF16, tag=f"ctxT{m}", name=f"ctxT{m}")
                for m in range(4)
            ]

            if reps != 1 and "qkv" not in phases:
                for m in range(4):
                    nc.gpsimd.memset(persist["qE"][m][0:HD, :], 0.125)
                    nc.gpsimd.memset(persist["qO"][m][HD:P, :], 0.125)
                    nc.gpsimd.memset(persist["kT2"][m][:], 0.125)
                for t in persist["v_sb"]:
                    nc.gpsimd.memset(t[:], 0.125)
            pre = {}
            if reps != 1 and "exp" not in phases and "ctx" in phases:
                for tk in range(8):
                    t = inp.tile([P, S], BF16, tag=f"pb{tk}")
                    nc.gpsimd.memset(t[:], 0.001)
                    pre[f"pb{tk}"] = t
            if reps != 1 and "ctx" not in phases and "out" in phases:
                for m in range(4):
                    nc.gpsimd.memset(persist["ctxT"][m][:], 0.125)

            # pin the exp activation-table set before the loop
            dummy = inp.tile([P, 8], F32, tag="dummy")
            nc.gpsimd.memset(dummy[:], 0.0)
            nc.scalar.activation(
                dummy[:], dummy[:], mybir.ActivationFunctionType.Exp
            )

            if reps == 1:
                _compute(tc, tiles, persist, out_d, tok_d, phases, pre)
            else:
                hints = (
                    mybir.EngineType.PE,
                    mybir.EngineType.DVE,
                    mybir.EngineType.Pool,
                )
                with tc.For_i(0, reps, 1, hint_engines=hints):
                    _compute(tc, tiles, persist, out_d, tok_d, phases, pre)

    nc.compile()
    return nc


def _dma_inputs(tc, inp, xt_d, wqt_d, wkt_d, wvt_d, wot_d):
    # interleave per contraction-chunk so the first qk accumulation chains
    # can start as soon as chunk 0 lands instead of waiting for whole
    # tensors
    nc = tc.nc
    xt, wqt, wkt, wvt = [], [], [], []
    for kc in range(8):
        t = inp.tile([P, S], INPUT_DT, tag=f"xt{kc}", name=f"xt{kc}")
        nc.sync.dma_start(t[:], xt_d[kc * P : (kc + 1) * P, :])
        xt.append(t)
        for lst, d, tag in (
            (wqt, wqt_d, "wqt"), (wkt, wkt_d, "wkt"), (wvt, wvt_d, "wvt")
        ):
            t = inp.tile([P, HG], INPUT_DT, tag=f"{tag}{kc}", name=f"{tag}{kc}")
            nc.sync.dma_start(t[:], d[kc * P : (kc + 1) * P, :])
            lst.append(t)
    wot = []  # 4 x [128, 1024] bf16, rows = local c
    for cc in range(4):
        t = inp.tile([P, H], BF16, tag=f"wot{cc}", name=f"wot{cc}")
        nc.sync.dma_start(t[:], wot_d[cc * P : (cc + 1) * P, :])
        wot.append(t)
    return xt, wqt, wkt, wvt, wot


def _compute(tc, tiles, persist, out_d, tok_d=None, phases=ALL_PHASES, pre=None):
    pre = pre or {}
    nc = tc.nc
    xt, wqt, wkt, wvt, wot = tiles
    v_sb = persist["v_sb"]
    qE, qO = persist["qE"], persist["qO"]
    kT2, ctxT = persist["kT2"], persist["ctxT"]
    from contextlib import ExitStack

    do_qkv = "qkv" in phases
    do_attn = "attn" in phases
    do_exp = "exp" in phases
    do_ctx = "ctx" in phases and do_attn
    do_out = "out" in phases

    ctx_stk = ExitStack()
    with ctx_stk:
        probs_pool = ctx_stk.enter_context(tc.tile_pool(name="probs", bufs=24))
        small = ctx_stk.enter_context(tc.tile_pool(name="small", bufs=4))
        outsb = ctx_stk.enter_context(tc.tile_pool(name="outsb", bufs=4))
        # PSUM budget (8 banks):
        #   sc   [128,1024] x2 bufs = 4 banks (scores, head-serial dbuf)
        #   chain[128,512]  x2 bufs = 2 banks (qk chains, v, out)
        #   pch  [128,512]  x2 bufs = 2 banks (ctx accumulators)
        ps_sc = ctx_stk.enter_context(
            tc.tile_pool(name="ps_sc", bufs=2, space="PSUM")
        )
        ps_sm = ctx_stk.enter_context(
            tc.tile_pool(name="ps_sm", bufs=2, space="PSUM")
        )

        probs = [[None] * 8 for _ in range(NH_LOC)]
        if not do_exp:
            for h in range(NH_LOC):
                for tk in range(8):
                    probs[h][tk] = pre.get(f"pb{tk}")

        # ---- filler generators (pumped between attention steps) ---------
        def gen_qk(m):
            for w, is_q in ((wqt, True), (wkt, False)):
                tA = ps_sm.tile([P, HG], F32, tag="chain", name="ch_qk")
                tB = ps_sm.tile([P, HG], F32, tag="chain", name="ch_qk")
                for kc in range(8):
                    for th, t in ((0, tA), (1, tB)):
                        nc.tensor.matmul(
                            t[:],
                            w[kc][:, m * P : (m + 1) * P],
                            xt[kc][:, th * 512 : (th + 1) * 512],
                            start=(kc == 0),
                            stop=(kc == 7),
                        )
                    if kc % 2 == 1:
                        yield
                if is_q:
                    for th, t in ((0, tA), (1, tB)):
                        sl = slice(th * 512, (th + 1) * 512)
                        nc.vector.tensor_copy(qE[m][0:HD, sl], t[0:HD, :])
                        nc.scalar.activation(
                            qO[m][HD:P, sl], t[HD:P, :],
                            mybir.ActivationFunctionType.Copy,
                        )
                else:
                    for th, t in ((0, tA), (1, tB)):
                        sl = slice(th * 512, (th + 1) * 512)
                        nc.vector.tensor_copy(kT2[m][:, sl], t[:])
                yield

        def gen_v():
            for tci in range(8):
                ps = ps_sm.tile([P, HG], F32, tag="chain", name="ps_v")
                for kc in range(8):
                    nc.tensor.matmul(
                        ps[:],
                        xt[kc][:, tci * P : (tci + 1) * P],
                        wvt[kc][:],
                        start=(kc == 0),
                        stop=(kc == 7),
                    )
                    if kc % 4 == 3:
                        yield
                for h in range(NH_LOC):
                    src = ps[:, h * HD : (h + 1) * HD]
                    dst = v_sb[tci][:, h * P : h * P + HD]
                    if h % 2 == 0:
                        nc.vector.tensor_copy(dst, src)
                    else:
                        nc.scalar.activation(
                            dst, src, mybir.ActivationFunctionType.Copy
                        )
                yield

        def gen_ctx(h):
            m, hh = h // 2, h % 2
            pch = [
                ps_sm.tile([P, 512], F32, tag="pch", name="pch")
                for _ in range(2)
            ]
            for tk in range(8):
                for th in range(2):
                    nc.tensor.matmul(
                        pch[th][:],
                        v_sb[tk][:, h * P : (h + 1) * P],
                        probs[h][tk][:, th * 512 : (th + 1) * 512],
                        start=(tk == 0),
                        stop=(tk == 7),
                    )
                yield
            for th in range(2):
                rp = small.tile([HD, 512], F32, tag="recip", name="rp")
                nc.vector.reciprocal(rp[:], pch[th][64:128, :])
                nc.vector.tensor_tensor(
                    ctxT[m][hh * HD : (hh + 1) * HD, th * 512 : (th + 1) * 512],
                    pch[th][0:HD, :],
                    rp[:],
                    mybir.AluOpType.mult,
                )
            yield

        fillers = deque()
        if do_qkv:
            fillers.append(("qk1", gen_qk(1)))
            fillers.append(("v", gen_v()))
            fillers.append(("qk2", gen_qk(2)))
            fillers.append(("qk3", gen_qk(3)))
        done_fillers = set()
        ctx_tasks = deque()

        def pump_filler(n):
            for _ in range(n):
                while fillers:
                    name, g = fillers[0]
                    try:
                        next(g)
                        break
                    except StopIteration:
                        done_fillers.add(name)
                        fillers.popleft()
                else:
                    return

        def drain_filler(name):
            while fillers and name not in done_fillers:
                pump_filler(1)

        def pump_ctx(n):
            for _ in range(n):
                while ctx_tasks:
                    try:
                        next(ctx_tasks[0])
                        break
                    except StopIteration:
                        ctx_tasks.popleft()
                else:
                    return

        # ---- lead-in: first qk pair ------------------------------------
        if do_qkv:
            for _ in gen_qk(0):
                pass

        # ---- attention: pair loop with interleaved ctx + filler --------
        if do_attn:
            for m in range(4):
                if do_qkv and m >= 1:
                    drain_filler("v")
                    drain_filler(f"qk{m}")
                for tk in range(8):
                    for h, qpad in ((2 * m, qE[m]), (2 * m + 1, qO[m])):
                        ps = ps_sc.tile([P, S], F32, tag="sc", name="sc")
                        for th in range(2):
                            nc.tensor.matmul(
                                ps[:, th * 512 : (th + 1) * 512],
                                kT2[m][:, tk * P : (tk + 1) * P],
                                qpad[:, th * 512 : (th + 1) * 512],
                                start=True,
                                stop=True,
                            )
                        if do_exp:
                            pb = probs_pool.tile(
                                [P, S], BF16, tag="pb", name="pb"
                            )
                            nc.scalar.activation(
                                pb[:], ps[:],
                                mybir.ActivationFunctionType.Exp,
                                scale=0.125,
                            )
                            probs[h][tk] = pb
                        pump_ctx(1)
                    pump_filler(5 if m == 0 else 2)
                if do_ctx:
                    ctx_tasks.extend([gen_ctx(2 * m), gen_ctx(2 * m + 1)])
                pump_ctx(2)

        # drain leftovers
        pump_filler(1000)
        pump_ctx(1000)

        # ---- output projection -----------------------------------------
        if do_out:
            for tci in range(8):
                oa = outsb.tile([P, H], F32, tag="oa", name="oa")
                for ho in range(2):
                    ps = ps_sm.tile([P, 512], F32, tag="pch", name="ps_o")
                    for cc in range(4):
                        nc.tensor.matmul(
                            ps[:],
                            ctxT[cc][:, tci * P : (tci + 1) * P],
                            wot[cc][:, ho * 512 : (ho + 1) * 512],
                            start=(cc == 0),
                            stop=(cc == 3),
                        )
                    dst = oa[:, ho * 512 : (ho + 1) * 512]
                    if ho == 0:
                        nc.vector.tensor_copy(dst, ps[:])
                    else:
                        nc.scalar.activation(
                            dst, ps[:], mybir.ActivationFunctionType.Copy
                        )
                nc.sync.dma_start(
                    out_d[tci * P : (tci + 1) * P, :], oa[:]
                )

        if tok_d is not None:
            tk_t = small.tile([1, 4], F32, tag="tok")
            nc.gpsimd.memset(tk_t[:], 0.0)
            nc.sync.dma_start(tok_d[:], tk_t[:])


def _get_nc():
    if "nc" not in _CACHE:
        _CACHE["nc"] = _build_graph()
    return _CACHE["nc"]


def kernel(x, mask, Wq, bq, Wk, bk, Wv, bv, Wo, bo):
    x = np.asarray(x, dtype=np.float32)
    Wq = np.asarray(Wq, dtype=np.float32)
    Wk = np.asarray(Wk, dtype=np.float32)
    Wv = np.asarray(Wv, dtype=np.float32)
    Wo = np.asarray(Wo, dtype=np.float32)

    nc = _get_nc()
    bf = ml_dtypes.bfloat16
    in_maps = []
    for c in range(8):
        b, g = c // 2, c % 2
        sl = slice(g * HG, (g + 1) * HG)
        in_maps.append(
            {
                "xt": np.ascontiguousarray(x[b].T.astype(bf)),
                "wqt": np.ascontiguousarray(Wq[sl, :].T.astype(bf)),
                "wkt": np.ascontiguousarray(Wk[sl, :].T.astype(bf)),
                "wvt": np.ascontiguousarray(Wv[sl, :].T.astype(bf)),
                "wot": np.ascontiguousarray(Wo[:, sl].T.astype(bf)),
            }
        )
    res = run_bass_kernel_spmd(
        nc, in_maps, core_ids=list(range(8)), **_CACHE.get("run_kwargs", {})
    )
    _CACHE["last_result"] = res
    outs = [res.results[c]["out_p"] for c in range(8)]
    return np.stack(
        [outs[2 * b] + outs[2 * b + 1] for b in range(4)]
    ).astype(np.float32)
